# revision 1
# baseline (speedup 1.0000x reference)
"""Trainium2 Bass kernel for causal multi-head self-attention.

Problem: x[4,2048,1024] @ w_qkv[1024,3072] -> causal MHA (16 heads, d=64)
         -> @ w_proj[1024,1024].

Sharding (8 cores): core c handles batch b=c//2 and head-group g=c%2
(8 of 16 heads). Each core computes QKV for its heads, causal attention,
and a partial output projection over its heads' w_proj rows (transposed
layout [C, T]). Host sums the two partials per batch and transposes back.

v2 layout notes (vs baseline):
- All inputs stream in as bf16 (host-converted): halves DMA, enables the
  PE's fast-weight-load path, and lifts the f32r N>=256 restriction so
  causal windows are exact multiples of 128.
- exp alternates between ScalarE ACTIVATE and a DVE Schraudolph bit-exp
  (tensor_scalar mult+add -> int32, bitcast to f32; ~3% P error on half
  the tiles, cancels through softmax normalization).
- 1/l uses reciprocal_approx_fast (single DVE op) instead of the 3.3us
  iterative reciprocal.
- Next pair's Q/K matmuls and the final projection are emitted
  interleaved into the attention slot loop (PE is in-order; emission
  order = execution order), filling exp-wait bubbles.
- PSUM: st 2 bufs x 2 banks, avA/avB 1 bank each, pq/yp ring 2 banks.
"""

import numpy as np

import concourse.mybir as mybir
import concourse.tile as tile
from concourse import bacc, bass_utils

F32 = mybir.dt.float32
I32 = mybir.dt.int32
BF16 = mybir.dt.bfloat16
AF = mybir.ActivationFunctionType
ALU = mybir.AluOpType
NP_ = 128  # partitions

# Schraudolph bit-exp at bf16 granularity: int16 bits of bf16(exp(x*scale))
# = round(x * scale * 2^7/ln2 + (127*2^7 - C))
_EXP_A = 12102203.161561485 / 65536.0  # 2^7 / ln 2
_EXP_B = 1064992558.5 / 65536.0 + 0.5  # (127*2^23 - 360658)/2^16 (+0.5 trunc guard)


def build_nc(T=2048, C=1024, HL=8, D=64, num_devices=8, debug=False, reps=1):
    """Build the per-core SPMD program. HL = local heads (must be even)."""
    HD = HL * D  # local qkv feature count
    CK = C // NP_  # contraction chunks over C
    TB = 512  # t/q block
    NTB = T // TB
    KT = NP_  # key tile
    NPAIR = HL // 2
    YC = C // NP_  # y-column tiles
    NKT = T // KT

    nc = bacc.Bacc(
        "TRN2", target_bir_lowering=False, debug=debug, num_devices=num_devices
    )
    xt_d = nc.dram_tensor("xt", [C, T], BF16, kind="ExternalInput")
    wq_d = nc.dram_tensor("wq", [C, HD], BF16, kind="ExternalInput")
    wk_d = nc.dram_tensor("wk", [C, HD], BF16, kind="ExternalInput")
    wv_d = nc.dram_tensor("wv", [C, HD], BF16, kind="ExternalInput")
    wp_d = nc.dram_tensor("wp", [HD, C], BF16, kind="ExternalInput")
    yt_d = nc.dram_tensor("yt", [C, T], F32, kind="ExternalOutput")

    scale = 1.0 / np.sqrt(D)

    with tile.TileContext(nc) as tc:
        with (
            tc.tile_pool(name="psMM", bufs=2, space="PSUM") as psMM,
            tc.tile_pool(name="psST", bufs=2, space="PSUM") as psST,
            tc.tile_pool(name="psAV", bufs=1, space="PSUM") as psAV,
            tc.tile_pool(name="res", bufs=1) as res,
            tc.tile_pool(name="pp", bufs=3) as pp,
            tc.tile_pool(name="work", bufs=2) as work,
            tc.tile_pool(name="wqk", bufs=4) as wqk,
            tc.tile_pool(name="xpool", bufs=1) as xpool,
            tc.tile_pool(name="wppool", bufs=1) as wppool,
            tc.tile_pool(name="ypool", bufs=4) as ypool,
        ):
            for _rep in range(reps):
                filler = []  # deferred emission chunks (lambdas)
                fpos = [0]

                def drain(n):
                    e = min(fpos[0] + n, len(filler))
                    while fpos[0] < e:
                        filler[fpos[0]]()
                        fpos[0] += 1

                def dma_w(tag, src_d, p):
                    w = wqk.tile([NP_, CK, NP_], BF16, tag=tag, name=tag)
                    nc.sync.dma_start(
                        w[:],
                        src_d[:, p * NP_ : (p + 1) * NP_].rearrange(
                            "(c pp) f -> pp c f", pp=NP_
                        ),
                    )
                    return w

                # ---- input DMA: pair-0 weights first, then x, wv, rest
                ws = [
                    (dma_w("wqp", wq_d, 0), dma_w("wkp", wk_d, 0)),
                ]
                xt_r = xt_d.rearrange("(c p) t -> p c t", p=NP_)
                xt = [
                    xpool.tile([NP_, T], BF16, tag=f"xt{c}", name="xc")
                    for c in range(CK)
                ]
                # quarter-T DMA slices: the first Q/K chunk + V0-3 only
                # need x columns [0:512] (~1MB), so the PE starts ~4us
                # earlier; wv lands right after the first quarter so the
                # inline V0-3 tiles aren't DMA-gated.
                wv_r = wv_d.rearrange("(c p) f -> p c f", p=NP_)
                wv = xpool.tile([NP_, CK, HD], BF16, tag="wv", name="wv")
                TQ = T // 4
                for h in range(4):
                    for c in range(CK):
                        nc.sync.dma_start(
                            xt[c][:, h * TQ : (h + 1) * TQ],
                            xt_r[:, c, h * TQ : (h + 1) * TQ],
                        )
                    if h == 0:
                        nc.sync.dma_start(wv[:], wv_r[:])
                for p in range(1, NPAIR):
                    ws.append((dma_w("wqp", wq_d, p), dma_w("wkp", wk_d, p)))
                wp = wppool.tile([NP_, HD // NP_, C], BF16)
                nc.sync.dma_start(
                    wp[:], wp_d.rearrange("(m pp) c -> pp m c", pp=NP_)
                )

                def emit_qkv_mms(p, tb, w_i):
                    """One tb-column of Q or K for pair p: 8 MMs into a pq
                    tile; returns the tile for the (lagged) cast item."""
                    w = ws[p][w_i]
                    pq = psMM.tile([NP_, TB], F32, tag="mm", name="pq")
                    for c in range(CK):
                        nc.tensor.matmul(
                            pq[:],
                            w[:, c, :],
                            xt[c][:, tb * TB : (tb + 1) * TB],
                            start=(c == 0),
                            stop=(c == CK - 1),
                            skip_group_check=True,
                        )
                    return pq

                def emit_qkv_cast(pq, tb, dst):
                    nc.scalar.copy(dst[:, tb * TB : (tb + 1) * TB], pq[:])

                def emit_qkv_chunk(p, tb, w_i, qt, ktt):
                    pq = emit_qkv_mms(p, tb, w_i)
                    emit_qkv_cast(pq, tb, (qt, ktt)[w_i])

                def lag1(pairs):
                    """pairs: list of (mm_thunk, cast_thunk_maker). Emission
                    order [mm0, mm1, cast0, mm2, cast1, ..., castN]: a cast
                    reaches its engine queue only after the NEXT chunk's MMs,
                    so it never blocks the queue waiting on PE progress."""
                    items = []
                    pending = []

                    def run_mm(mm, cast_maker, pend=pending):
                        pend.append(cast_maker(mm()))

                    def run_cast(pend=pending):
                        pend.pop(0)()

                    for k, (mm, cm) in enumerate(pairs):
                        items.append(
                            lambda mm=mm, cm=cm: run_mm(mm, cm)
                        )
                        if k >= 1:
                            items.append(run_cast)
                    if pairs:
                        items.append(run_cast)
                    return items

                def new_qk(p):
                    qt = work.tile([NP_, T], BF16, tag="qt", name="qt")
                    ktt = work.tile([NP_, T], BF16, tag="ktt", name="ktt")
                    return qt, ktt

                def emit_v_mms(kt, vt):
                    nc.gpsimd.memset(vt[:, :, D : D + 1], 1.0)
                    pv = psMM.tile([NP_, HD], F32, tag="mm", name="pv")
                    for c in range(CK):
                        nc.tensor.matmul(
                            pv[:],
                            xt[c][:, kt * KT : (kt + 1) * KT],
                            wv[:, c, :],
                            start=(c == 0),
                            stop=(c == CK - 1),
                            skip_group_check=True,
                        )
                    return pv

                def emit_v_cast(pv, vt):
                    nc.vector.tensor_copy(
                        vt[:, :, 0:D],
                        pv[:].rearrange("p (h d) -> p h d", d=D),
                    )

                slot_idx = [0]
                pace = [1]  # slots per filler drain (1 = every slot)

                deferred_norm = []

                def emit_attn(p, qt, ktt, vts, aot, on_qb_done=None):
                    for qb in range(NTB):
                        nkt = (qb + 1) * (TB // KT)
                        avA = psAV.tile([D + 1, TB], F32, tag="avA", name="avA")
                        avB = psAV.tile([D + 1, TB], F32, tag="avB", name="avB")

                        def emit_pv(kti, moff, pt):
                            first, last = kti == 0, kti == nkt - 1
                            for i, av in ((0, avA), (1, avB)):
                                nc.tensor.matmul(
                                    av[:, moff:TB],
                                    vts[kti][:, 2 * p + i, :],
                                    pt[:, i, moff:TB],
                                    start=first,
                                    stop=last,
                                    skip_group_check=True,
                                )

                        pend_pv = []  # two-slot PV skew behind scores/exp
                        for kti in range(nkt):
                            if kti == 2 and deferred_norm:
                                deferred_norm.pop(0)()
                            j = kti - qb * (TB // KT)
                            moff = 128 * j if j >= 0 else 0
                            st = psST.tile([NP_, 2, TB], F32, tag="st", name="st")
                            for i in range(2):
                                nc.tensor.matmul(
                                    st[:, i, moff:TB],
                                    ktt[
                                        i * D : (i + 1) * D,
                                        kti * KT : (kti + 1) * KT,
                                    ],
                                    qt[
                                        i * D : (i + 1) * D,
                                        qb * TB + moff : (qb + 1) * TB,
                                    ],
                                    start=True,
                                    stop=True,
                                )
                            use_act = slot_idx[0] % 2 == 0 or kti == 0
                            slot_idx[0] += 1
                            if use_act:
                                pt = pp.tile(
                                    [NP_, 2, TB], BF16, tag="pta", name="pta",
                                    bufs=4,
                                )
                                nc.scalar.activation(
                                    pt[:, :, moff:TB], st[:, :, moff:TB],
                                    AF.Exp, scale=scale,
                                )
                            else:
                                pti = pp.tile(
                                    [NP_, 2, TB], mybir.dt.int16, tag="ptv",
                                    name="ptv", bufs=4,
                                )
                                nc.vector.tensor_scalar(
                                    pti[:, :, moff:TB], st[:, :, moff:TB],
                                    _EXP_A * scale, _EXP_B,
                                    ALU.mult, ALU.add,
                                )
                                pt = pti.bitcast(BF16)
                            if j >= 0:
                                nc.gpsimd.affine_select(
                                    out=pt[:, :, 128 * j : 128 * j + 128],
                                    in_=pt[:, :, 128 * j : 128 * j + 128],
                                    compare_op=ALU.is_ge,
                                    fill=0.0,
                                    base=0,
                                    pattern=[[0, 2], [1, 128]],
                                    channel_multiplier=-1,
                                )
                            pend_pv.append((kti, moff, pt))
                            if len(pend_pv) > 1:
                                emit_pv(*pend_pv.pop(0))
                            if slot_idx[0] % pace[0] == 0:
                                drain(1)
                        while pend_pv:
                            emit_pv(*pend_pv.pop(0))
                        # normalization part A (at boundary): araw copies on
                        # ACT + DVE in parallel — frees the av banks fast so
                        # the next qb's first PV can start.
                        araws = []
                        for i, av in ((0, avA), (1, avB)):
                            araw = pp.tile(
                                [D + 1, TB], F32, tag=f"araw{i}", name="araw",
                                bufs=3,
                            )
                            if i == 0:
                                nc.scalar.copy(araw[:], av[:])
                            else:
                                nc.vector.tensor_copy(araw[:], av[:])
                            araws.append(araw)

                        # part B (deferred ~3 slots into the next qb): the
                        # lrow/broadcast/recip/mul tail, kept off the DVE
                        # queue head so the next qb's bit-exps aren't stuck
                        # behind it.
                        def norm_tail(qb=qb, araws=araws):
                            for i in range(2):
                                araw = araws[i]
                                lrow = pp.tile([1, TB], F32, tag="lrow",
                                               name="lrow", bufs=2)
                                nc.vector.tensor_copy(
                                    lrow[:], araw[D : D + 1, :]
                                )
                                bca = pp.tile([D, TB], F32, tag="bca",
                                              name="bca", bufs=2)
                                nc.gpsimd.partition_broadcast(bca[:], lrow[:])
                                rec = pp.tile([D, TB], F32, tag="rec",
                                              name="rec", bufs=2)
                                nc.vector.reciprocal_approx_fast(rec[:], bca[:])
                                nc.vector.tensor_mul(
                                    aot[i * D : (i + 1) * D,
                                        qb * TB : (qb + 1) * TB],
                                    araw[0:D, :],
                                    rec[:],
                                )
                            if on_qb_done is not None:
                                on_qb_done(qb)
                            drain(2)

                        deferred_norm.append(norm_tail)

                def emit_proj_mms(yc, tb, aots):
                    yp = psMM.tile([NP_, TB], F32, tag="mm", name="yp")
                    for m in range(HD // NP_):
                        nc.tensor.matmul(
                            yp[:],
                            wp[:, m, yc * NP_ : (yc + 1) * NP_],
                            aots[m][:, tb * TB : (tb + 1) * TB],
                            start=(m == 0),
                            stop=(m == HD // NP_ - 1),
                            skip_group_check=True,
                        )
                    return yp

                def emit_proj_out(yp, yc, tb):
                    ysb = ypool.tile([NP_, TB], F32, tag="y", name="ysb")
                    nc.scalar.copy(ysb[:], yp[:])
                    nc.sync.dma_start(
                        yt_d[yc * NP_ : (yc + 1) * NP_, tb * TB : (tb + 1) * TB],
                        ysb[:],
                    )

                def emit_proj_tile(yc, tb, aots):
                    emit_proj_out(emit_proj_mms(yc, tb, aots), yc, tb)

                # ---- main schedule: attention starts right after QK(tb0)
                # + the first 4 V tiles; everything else (rest of pair-0 QK,
                # V tiles 4-15, next pairs' QK, proj) drains as filler.
                vts = [
                    res.tile([NP_, HL, D + 1], BF16, tag=f"vt{kt}", name="vt")
                    for kt in range(NKT)
                ]
                qt0, ktt0 = new_qk(0)
                for w_i in range(2):
                    emit_qkv_chunk(0, 0, w_i, qt0, ktt0)
                for kt in range(4):
                    emit_v_cast(emit_v_mms(kt, vts[kt]), vts[kt])
                aots = [
                    res.tile([NP_, T], BF16, tag=f"aot{p}", name="aot")
                    for p in range(NPAIR)
                ]

                def qk_items(p1, tbs, q, k):
                    return lag1([
                        (
                            (lambda p1=p1, tb=tb, w_i=w_i:
                             emit_qkv_mms(p1, tb, w_i)),
                            (lambda pq, tb=tb, w_i=w_i, q=q, k=k:
                             (lambda: emit_qkv_cast(pq, tb, (q, k)[w_i]))),
                        )
                        for tb in tbs
                        for w_i in range(2)
                    ])

                cur = (qt0, ktt0)
                for p in range(NPAIR):
                    on_qb_done = None
                    if p == 0:
                        qtn, kttn = new_qk(1)

                        def v_lag(kts):
                            return lag1([
                                (
                                    (lambda kt=kt: emit_v_mms(kt, vts[kt])),
                                    (lambda pv, kt=kt:
                                     (lambda: emit_v_cast(pv, vts[kt]))),
                                )
                                for kt in kts
                            ])

                        def qk_nolag(tb):
                            # [mm_q, cast_q, mm_k, cast_k] — q-cast lands one
                            # item earlier than lag1, meeting the emission
                            # deadline for the next qb's first scores.
                            items = []
                            for w_i in range(2):
                                def mmcast(tb=tb, w_i=w_i):
                                    pq = emit_qkv_mms(0, tb, w_i)
                                    return lambda: emit_qkv_cast(
                                        pq, tb, (qt0, ktt0)[w_i])
                                items.append(mmcast)
                            out = []
                            pend = []
                            for it in items:
                                out.append(lambda it=it, pend=pend:
                                           pend.append(it()))
                                out.append(lambda pend=pend: pend.pop(0)())
                            return out

                        # ordering satisfies emission deadlines (one drain
                        # per slot, +2 per deferred norm tail): qb1 needs
                        # QKtb1 + V4-7 early, qb2 needs QKtb2 by item 14 and
                        # V8-11, qb3 needs QKtb3 + V12-15.
                        filler = (
                            qk_items(0, [1], qt0, ktt0)
                            + v_lag(range(4, 8))
                            + qk_nolag(2)
                            + v_lag(range(8, 12))
                            + qk_nolag(3)
                            + v_lag(range(12, 16))
                            + qk_items(1, range(NTB), qtn, kttn)
                        )
                    elif p + 1 < NPAIR:
                        qtn, kttn = new_qk(p + 1)
                        tbs = [0, 1] if p + 1 == NPAIR - 1 else range(NTB)
                        filler = qk_items(p + 1, tbs, qtn, kttn)
                        pace[0] = 2
                    else:
                        filler = qk_items(p, [2], cur[0], cur[1]) + qk_items(
                            p, [3], cur[0], cur[1]
                        )
                        pace[0] = 1

                        def on_qb_done(qb):
                            # aot[:, qb] of ALL pairs is now emitted; proj
                            # tiles for tb=qb become safe to emit.
                            filler.extend(lag1([
                                (
                                    (lambda yc=yc, tb=qb:
                                     emit_proj_mms(yc, tb, aots)),
                                    (lambda yp, yc=yc, tb=qb:
                                     (lambda: emit_proj_out(yp, yc, tb))),
                                )
                                for yc in range(YC)
                            ]))
                    fpos = [0]
                    emit_attn(p, cur[0], cur[1], vts, aots[p], on_qb_done)
                    if p + 1 == NPAIR:
                        while deferred_norm:  # last qb's norm + proj gating
                            deferred_norm.pop(0)()
                    drain(len(filler))  # anything left over
                    if p + 1 < NPAIR:
                        cur = (qtn, kttn)

    nc.compile()
    return nc


_NC_CACHE = {}


def _get_nc():
    if "nc" not in _NC_CACHE:
        _NC_CACHE["nc"] = build_nc()
    return _NC_CACHE["nc"]


def make_in_maps(x, w_qkv, w_proj):
    import ml_dtypes

    B, T, C = x.shape
    H = 16
    D = C // H
    bf = ml_dtypes.bfloat16
    in_maps = []
    for core in range(8):
        b, g = core // 2, core % 2
        h0 = g * 8
        xT = np.ascontiguousarray(x[b].T).astype(bf)
        wq = np.ascontiguousarray(w_qkv[:, h0 * D : (h0 + 8) * D]).astype(bf)
        wk = np.ascontiguousarray(
            w_qkv[:, C + h0 * D : C + (h0 + 8) * D]
        ).astype(bf)
        wv = np.ascontiguousarray(
            w_qkv[:, 2 * C + h0 * D : 2 * C + (h0 + 8) * D]
        ).astype(bf)
        wp = np.ascontiguousarray(w_proj[g * 512 : (g + 1) * 512, :]).astype(bf)
        in_maps.append({"xt": xT, "wq": wq, "wk": wk, "wv": wv, "wp": wp})
    return in_maps


def kernel(x, w_qkv, w_proj):
    x = np.asarray(x, dtype=np.float32)
    w_qkv = np.asarray(w_qkv, dtype=np.float32)
    w_proj = np.asarray(w_proj, dtype=np.float32)
    nc = _get_nc()
    in_maps = make_in_maps(x, w_qkv, w_proj)
    res = bass_utils.run_bass_kernel_spmd(nc, in_maps, core_ids=list(range(8)))
    B, T, C = x.shape
    y = np.empty((B, T, C), np.float32)
    for b in range(B):
        yt = res.results[2 * b]["yt"] + res.results[2 * b + 1]["yt"]
        y[b] = yt.T
    return y

